# revision 3
# baseline (speedup 1.0000x reference)
"""Trainium2 Bass kernel for nn_Kernel_11344467299061915904_53472342835846.

Reference computation (N=16, C=128, H=64, W=64, S=H*W=4096):
    t1[n,c,k,i,j] = x[n,c, i+2k-6, j]        (zero-padded in H)
    t3 = p3[c,k] * p2[c,j] * t1
    t8[n,c',(c2,k)] = sum_s x[n,c',s] t3[n,(c2,k),s] / sqrt(S)
    t7 = conv1x7(x, w7)                       (dense, 896 out channels)
    t9 = (t8 @ t7) / sqrt(7C)
    t6 = depthwise H-conv taps {-3,0,3} of roll(p4*x, 1, axis=W)
    out = t9 - t6

Restructured to cut FLOPs ~6.5x: t9 = sum_sft (t8 @ W7_sft) @ X_sft, so the
dense conv t7 is never materialized.  The H-shifts of t1 are +/-{0,2,4,6}
rows = multiples of 128 elements in (s, c) layout since 2*W = 128, so t8
becomes 32 chunk-matmuls against a block-shifted window of the transposed
input.  t6 is folded into the t9 PSUM accumulation as negated-diagonal
matmuls.  Data-parallel over batch: 2 samples per NeuronCore on 8 cores.

Host-side work is layout-only (transpose/pad/permute; zero FLOPs) plus
O(C*K) parameter prep; all O(N*C*S) arithmetic runs on device.
"""

import math

import numpy as np

N, C, H, W = 16, 128, 64, 64
S = H * W            # 4096
NB = S // 128        # 32 s-chunks of 128
NBP = NB + 6         # 38 blocks incl 3 zero pad blocks each side
PER_CORE = 2         # samples per NeuronCore
N_CORES = 8

_COMPILED = None


def _build_nc():
    import concourse.bass as bass
    import concourse.mybir as mybir
    import concourse.tile as tile
    from concourse import bacc

    f32 = mybir.dt.float32
    f32r = mybir.dt.float32r

    nc = bacc.Bacc("TRN2", target_bir_lowering=False, debug=False)

    # Per-core inputs (2 samples each), layouts pre-marshaled on host.
    xpad_d = nc.dram_tensor("xpad", [PER_CORE, C, H, W + 6], f32r, kind="ExternalInput").ap()
    xtp_d = nc.dram_tensor("xtp", [PER_CORE, 128, NBP, 128], f32r, kind="ExternalInput").ap()
    p2t_d = nc.dram_tensor("p2t", [128, 128], f32, kind="ExternalInput").ap()
    p4p_d = nc.dram_tensor("p4p", [C, H, W], f32, kind="ExternalInput").ap()
    w7r_d = nc.dram_tensor("w7r", [C, 7, 7, C], f32r, kind="ExternalInput").ap()
    scl_d = nc.dram_tensor("scl", [C, 7], f32, kind="ExternalInput").ap()
    dng_d = nc.dram_tensor("dng", [3, C, C], f32r, kind="ExternalInput").ap()
    out_d = nc.dram_tensor("out", [PER_CORE, C, S], f32, kind="ExternalOutput").ap()

    with tile.TileContext(nc) as tc:
        with (
            tc.tile_pool(name="consts", bufs=1) as consts,
            tc.tile_pool(name="xin", bufs=2) as xin,
            tc.tile_pool(name="xtr", bufs=1) as xtr,
            tc.tile_pool(name="ytr", bufs=1) as ytr,
            tc.tile_pool(name="small", bufs=1) as small,
            tc.tile_pool(name="ostage", bufs=3) as ostage,
            tc.tile_pool(name="pt8", bufs=2, space="PSUM") as pt8_pool,
            tc.tile_pool(name="pa", bufs=2, space="PSUM") as pa_pool,
            tc.tile_pool(name="pt9", bufs=2, space="PSUM") as pt9_pool,
        ):
            # ---- constants (once per core) ----
            p2t = consts.tile([128, 128], f32, tag="p2t")
            nc.sync.dma_start(out=p2t, in_=p2t_d)
            p4p = consts.tile([C, H, W], f32, tag="p4p")
            nc.sync.dma_start(out=p4p, in_=p4p_d)
            w7r = consts.tile([C, 7, 7, C], f32r, tag="w7r")
            nc.sync.dma_start(out=w7r, in_=w7r_d)
            scl = consts.tile([C, 7], f32, tag="scl")
            nc.sync.dma_start(out=scl, in_=scl_d)
            dng = consts.tile([C, 3, C], f32r, tag="dng")
            nc.sync.dma_start(out=dng, in_=dng_d.rearrange("r p c -> p r c"))

            zcst = consts.tile([C, 3 * W], f32, tag="zcst")
            nc.vector.memset(zcst, 0.0)

            # shared across both samples: scaled transposed t8 and A matrices
            t8ts = small.tile([C, 7, PER_CORE, C], f32r, tag="t8ts")
            a_sb = small.tile([C, 7, PER_CORE, C], f32r, tag="a_sb")

            xpads = []
            t5ps = []

            # ---- per-sample: load, gate, t8 ----
            for ns in range(PER_CORE):
                xpad = xin.tile([C, H, W + 6], f32r, tag="xpad")
                nc.sync.dma_start(out=xpad, in_=xpad_d[ns])
                xpads.append(xpad)

                xtrev = xtr.tile([128, NBP, 128], f32r, tag="xtrev")
                nc.sync.dma_start(out=xtrev, in_=xtp_d[ns])

                # t5p: H-padded roll(p4*x, 1, axis=W); rows [3,67) hold data
                t5p = xin.tile([C, H + 6, W], f32r, tag="t5p")
                nc.vector.tensor_copy(t5p[:, 0:3, :], zcst.rearrange("p (a b) -> p a b", a=3))
                nc.vector.tensor_copy(t5p[:, H + 3:H + 6, :], zcst.rearrange("p (a b) -> p a b", a=3))
                # t5[c,i,j] = p4[c,i,j-1]*x[c,i,j-1]  (j>=1);  x[.,j] = xpad[., 3+j]
                nc.vector.tensor_mul(t5p[:, 3:3 + H, 1:W], xpad[:, :, 3:3 + W - 1].bitcast(f32), p4p[:, :, 0:W - 1])
                nc.vector.tensor_mul(t5p[:, 3:3 + H, 0:1], xpad[:, :, 2 + W:3 + W].bitcast(f32), p4p[:, :, W - 1:W])
                t5ps.append(t5p)

                # Yt blocks: gated transpose  Yt[m] = Xt[m] * P2T
                # xtrev block b holds X^T chunk m = 34-b  (b in [3,35))
                yt = ytr.tile([128, NB, 128], f32r, tag="yt")
                for m in range(NB):
                    nc.vector.tensor_mul(yt[:, m, :], xtrev[:, 34 - m, :].bitcast(f32), p2t)

                # t8: PT8[c2, d*128+c'] += Yt[mp].T @ XtrevBlocks[31-mp+d], d=0..6
                pt8a = pt8_pool.tile([128, 512], f32, tag="pt8a")
                pt8b = pt8_pool.tile([128, 384], f32, tag="pt8b")
                for mp in range(NB):
                    lhsT = yt[:, mp, :]
                    nc.tensor.matmul(
                        pt8a, lhsT, xtrev[:, 31 - mp:35 - mp, :],
                        start=(mp == 0), stop=(mp == NB - 1),
                    )
                    nc.tensor.matmul(
                        pt8b, lhsT, xtrev[:, 35 - mp:38 - mp, :],
                        start=(mp == 0), stop=(mp == NB - 1),
                    )

                # scaled copies into shared T8Ts[c2, d, ns, c']
                for d in range(7):
                    src = pt8a[:, 128 * d:128 * (d + 1)] if d < 4 else pt8b[:, 128 * (d - 4):128 * (d - 3)]
                    nc.vector.tensor_scalar_mul(t8ts[:, d, ns, :], src, scl[:, d:d + 1])

            # ---- A phase (both samples batched: N=256) ----
            # A_sft^T[c'', (ns, c')] = sum_k w7r[:, k, sft, :].T @ T8Ts[:, k, :, :]
            for sft in range(7):
                pa = pa_pool.tile([128, PER_CORE * 128], f32, tag="pa")
                for k in range(7):
                    nc.tensor.matmul(
                        pa, w7r[:, k, sft, :],
                        t8ts[:, k, :, :],
                        start=(k == 0), stop=(k == 6),
                    )
                nc.vector.tensor_copy(a_sb[:, sft, :, :], pa)

            # ---- t9 phase (+ folded -t6) ----
            for ns in range(PER_CORE):
                xpad, t5p = xpads[ns], t5ps[ns]
                for j8 in range(8):
                    pt9 = pt9_pool.tile([128, 512], f32, tag="pt9")
                    for sft in range(7):
                        nc.tensor.matmul(
                            pt9, a_sb[:, sft, ns, :],
                            xpad[:, 8 * j8:8 * j8 + 8, sft:sft + W],
                            start=(sft == 0), stop=False,
                        )
                    for r in range(3):
                        nc.tensor.matmul(
                            pt9, dng[:, r, :],
                            t5p[:, 8 * j8 + 3 * r:8 * j8 + 3 * r + 8, :],
                            start=False, stop=(r == 2),
                        )
                    osb = ostage.tile([128, 512], f32, tag="osb")
                    nc.vector.tensor_copy(osb, pt9)
                    nc.sync.dma_start(out=out_d[ns, :, 512 * j8:512 * (j8 + 1)], in_=osb)

    nc.compile()
    return nc


def _prep_core_inputs(xs, p2, p3, p4, w6, w7):
    """Layout-only marshaling for one core's shard xs (PER_CORE,C,H,W)."""
    xs = np.ascontiguousarray(xs, dtype=np.float32)
    xpad = np.zeros((PER_CORE, C, H, W + 6), np.float32)
    xpad[:, :, :, 3:3 + W] = xs
    # transposed blocks, reversed order, 3 zero blocks each side
    xt = xs.reshape(PER_CORE, C, S).transpose(0, 2, 1).reshape(PER_CORE, NB, 128, C)
    xtp = np.zeros((PER_CORE, 128, NBP, 128), np.float32)
    xtp[:, :, 3:3 + NB, :] = xt[:, ::-1].transpose(0, 2, 1, 3)
    return {"xpad": xpad, "xtp": xtp}


def kernel(x, p2, p3, p4, w6, w7):
    global _COMPILED
    from concourse.bass_utils import run_bass_kernel_spmd

    if _COMPILED is None:
        _COMPILED = _build_nc()
    nc = _COMPILED

    x = np.ascontiguousarray(x, dtype=np.float32)
    p2 = np.asarray(p2, dtype=np.float32)
    p3 = np.asarray(p3, dtype=np.float32)
    p4 = np.asarray(p4, dtype=np.float32)
    w6 = np.asarray(w6, dtype=np.float32)
    w7 = np.asarray(w7, dtype=np.float32)

    # shared (replicated) parameter prep — O(C*K) host work + pure layout
    p2t = np.empty((128, 128), np.float32)            # P2T[p, c] = p2[c, p%64]
    p2row = p2[0, :, 0, 0, :]                          # (C, W)
    p2t[0:64] = p2row.T
    p2t[64:128] = p2row.T
    scl = (p3[0, :, :, 0, 0] / (math.sqrt(S) * math.sqrt(7 * C))).astype(np.float32)
    w7r = np.ascontiguousarray(
        w7[:, :, 0, :].reshape(C, 7, C, 7).transpose(0, 1, 3, 2)
    )                                                  # (c2, k, sft, c'')
    dng = np.zeros((3, C, C), np.float32)
    for r in range(3):
        np.fill_diagonal(dng[r], -w6[:, 0, r, 0])
    p4p = np.ascontiguousarray(p4[0])

    shared = {"p2t": p2t, "p4p": p4p, "w7r": w7r, "scl": scl, "dng": dng}
    in_maps = []
    for i in range(N_CORES):
        m = _prep_core_inputs(x[PER_CORE * i:PER_CORE * (i + 1)], p2, p3, p4, w6, w7)
        m.update(shared)
        in_maps.append(m)

    res = run_bass_kernel_spmd(nc, in_maps, list(range(N_CORES)))
    out = np.concatenate([res.results[i]["out"] for i in range(N_CORES)], axis=0)
    return out.reshape(N, C, H, W)


# revision 7
# speedup vs baseline: 1.1948x; 1.1948x over previous
"""Trainium2 Bass kernel for nn_Kernel_11344467299061915904_53472342835846.

Reference computation (N=16, C=128, H=64, W=64, S=H*W=4096):
    t1[n,c,k,i,j] = x[n,c, i+2k-6, j]        (zero-padded in H)
    t3 = p3[c,k] * p2[c,j] * t1
    t8[n,c',(c2,k)] = sum_s x[n,c',s] t3[n,(c2,k),s] / sqrt(S)
    t7 = conv1x7(x, w7)                       (dense, 896 out channels)
    t9 = (t8 @ t7) / sqrt(7C)
    t6 = depthwise H-conv taps {-3,0,3} of roll(p4*x, 1, axis=W)
    out = t9 - t6

Restructured to cut FLOPs ~6.5x: t9 = sum_sft (t8 @ W7_sft) @ X_sft, so the
dense conv t7 is never materialized.  The H-shifts of t1 are +/-{0,2,4,6}
rows = multiples of 128 elements in (s, c) layout since 2*W = 128, so t8
becomes 32 chunk-matmuls against a block-shifted window of the transposed
input.  t6 is folded into the t9 PSUM accumulation as negated-diagonal
matmuls.  Data-parallel over batch: 2 samples per NeuronCore on 8 cores.

Host-side work is layout-only (transpose/pad/permute; zero FLOPs) plus
O(C*K) parameter prep; all O(N*C*S) arithmetic runs on device.
"""

import math

import numpy as np

N, C, H, W = 16, 128, 64, 64
S = H * W            # 4096
NB = S // 128        # 32 s-chunks of 128
NBP = NB + 6         # 38 blocks incl 3 zero pad blocks each side
PER_CORE = 2         # samples per NeuronCore
N_CORES = 8

_COMPILED = None


def _build_nc():
    import concourse.bass as bass
    import concourse.mybir as mybir
    import concourse.tile as tile
    from concourse import bacc

    f32 = mybir.dt.float32
    f32r = mybir.dt.float32r

    nc = bacc.Bacc("TRN2", target_bir_lowering=False, debug=False)

    # Per-core inputs (2 samples each), layouts pre-marshaled on host.
    xpad_d = nc.dram_tensor("xpad", [PER_CORE, C, H, W + 6], f32r, kind="ExternalInput").ap()
    xtp_d = nc.dram_tensor("xtp", [PER_CORE, 128, NBP, 128], f32r, kind="ExternalInput").ap()
    p2t_d = nc.dram_tensor("p2t", [128, 128], f32, kind="ExternalInput").ap()
    p4p_d = nc.dram_tensor("p4p", [C, H, W], f32, kind="ExternalInput").ap()
    w7r_d = nc.dram_tensor("w7r", [C, 7, 7, C], f32r, kind="ExternalInput").ap()
    scl_d = nc.dram_tensor("scl", [C, 7], f32, kind="ExternalInput").ap()
    dng_d = nc.dram_tensor("dng", [3, C, C], f32r, kind="ExternalInput").ap()
    out_d = nc.dram_tensor("out", [PER_CORE, C, S], f32, kind="ExternalOutput").ap()

    with tile.TileContext(nc) as tc:
        with (
            tc.tile_pool(name="consts", bufs=1) as consts,
            tc.tile_pool(name="xin", bufs=2) as xin,
            tc.tile_pool(name="xtr", bufs=1) as xtr,
            tc.tile_pool(name="ytr", bufs=1) as ytr,
            tc.tile_pool(name="small", bufs=1) as small,
            tc.tile_pool(name="ostage", bufs=3) as ostage,
            tc.tile_pool(name="pt8", bufs=2, space="PSUM") as pt8_pool,
            tc.tile_pool(name="pa", bufs=2, space="PSUM") as pa_pool,
            tc.tile_pool(name="pt9", bufs=2, space="PSUM") as pt9_pool,
        ):
            # ---- constants (once per core) ----
            p2t = consts.tile([128, 128], f32, tag="p2t")
            nc.sync.dma_start(out=p2t, in_=p2t_d)
            p4p = consts.tile([C, H, W], f32, tag="p4p")
            nc.gpsimd.dma_start(out=p4p, in_=p4p_d)
            w7r = consts.tile([C, 7, 7, C], f32r, tag="w7r")
            nc.gpsimd.dma_start(out=w7r, in_=w7r_d)
            scl = consts.tile([C, 7], f32, tag="scl")
            nc.sync.dma_start(out=scl, in_=scl_d)
            dng = consts.tile([C, 3, C], f32r, tag="dng")
            nc.gpsimd.dma_start(out=dng, in_=dng_d.rearrange("r p c -> p r c"))

            zcst = consts.tile([C, 3 * W], f32, tag="zcst")
            nc.vector.memset(zcst, 0.0)

            # shared across both samples: scaled transposed t8 and A matrices
            t8ts = small.tile([C, 7, PER_CORE, C], f32r, tag="t8ts")
            a_sb = small.tile([C, 7, PER_CORE, C], f32r, tag="a_sb")

            xpads = []
            t5ps = []

            # ---- per-sample: load, gate, t8 ----
            for ns in range(PER_CORE):
                xtrev = xtr.tile([128, NBP, 128], f32r, tag="xtrev")
                nc.sync.dma_start(out=xtrev[:, 19:NBP, :], in_=xtp_d[ns, :, 19:NBP, :])
                nc.sync.dma_start(out=xtrev[:, 0:19, :], in_=xtp_d[ns, :, 0:19, :])

                xpad = xin.tile([C, H, W + 6], f32r, tag="xpad")
                nc.sync.dma_start(out=xpad, in_=xpad_d[ns])
                xpads.append(xpad)

                # t5p: H-padded roll(p4*x, 1, axis=W); rows [3,67) hold data
                t5p = xin.tile([C, H + 6, W], f32r, tag="t5p")
                nc.vector.tensor_copy(t5p[:, 0:3, :], zcst.rearrange("p (a b) -> p a b", a=3))
                nc.vector.tensor_copy(t5p[:, H + 3:H + 6, :], zcst.rearrange("p (a b) -> p a b", a=3))
                # t5[c,i,j] = p4[c,i,j-1]*x[c,i,j-1]  (j>=1);  x[.,j] = xpad[., 3+j]
                nc.vector.tensor_mul(t5p[:, 3:3 + H, 1:W], xpad[:, :, 3:3 + W - 1].bitcast(f32), p4p[:, :, 0:W - 1])
                nc.vector.tensor_mul(t5p[:, 3:3 + H, 0:1], xpad[:, :, 2 + W:3 + W].bitcast(f32), p4p[:, :, W - 1:W])
                t5ps.append(t5p)

                # Yt blocks: gated transpose  Yt[m] = Xt[m] * P2T
                # xtrev block b holds X^T chunk m = 34-b  (b in [3,35))
                yt = ytr.tile([128, NB, 128], f32r, tag="yt")
                for m in range(NB):
                    nc.vector.tensor_mul(yt[:, m, :], xtrev[:, 34 - m, :].bitcast(f32), p2t)

                # t8: PT8[c2, d*128+c'] += Yt[mp].T @ XtrevBlocks[31-mp+d], d=0..6
                pt8a = pt8_pool.tile([128, 512], f32, tag="pt8a")
                pt8b = pt8_pool.tile([128, 384], f32, tag="pt8b")
                for mp in range(NB):
                    lhsT = yt[:, mp, :]
                    nc.tensor.matmul(
                        pt8a, lhsT, xtrev[:, 31 - mp:35 - mp, :],
                        start=(mp == 0), stop=(mp == NB - 1),
                    )
                    nc.tensor.matmul(
                        pt8b, lhsT, xtrev[:, 35 - mp:38 - mp, :],
                        start=(mp == 0), stop=(mp == NB - 1),
                    )

                # scaled copies into shared T8Ts[c2, d, ns, c']
                for d in range(7):
                    src = pt8a[:, 128 * d:128 * (d + 1)] if d < 4 else pt8b[:, 128 * (d - 4):128 * (d - 3)]
                    nc.vector.tensor_scalar_mul(t8ts[:, d, ns, :], src, scl[:, d:d + 1])

            # ---- A phase (both samples batched: N=256) ----
            # A_sft^T[c'', (ns, c')] = sum_k w7r[:, k, sft, :].T @ T8Ts[:, k, :, :]
            for sft in range(7):
                pa = pa_pool.tile([128, PER_CORE * 128], f32, tag="pa")
                for k in range(7):
                    nc.tensor.matmul(
                        pa, w7r[:, k, sft, :],
                        t8ts[:, k, :, :],
                        start=(k == 0), stop=(k == 6),
                    )
                nc.vector.tensor_copy(a_sb[:, sft, :, :], pa)

            # ---- t9 phase (+ folded -t6) ----
            for ns in range(PER_CORE):
                xpad, t5p = xpads[ns], t5ps[ns]
                for j8 in range(8):
                    pt9 = pt9_pool.tile([128, 512], f32, tag="pt9")
                    for sft in range(7):
                        nc.tensor.matmul(
                            pt9, a_sb[:, sft, ns, :],
                            xpad[:, 8 * j8:8 * j8 + 8, sft:sft + W],
                            start=(sft == 0), stop=False,
                        )
                    for r in range(3):
                        nc.tensor.matmul(
                            pt9, dng[:, r, :],
                            t5p[:, 8 * j8 + 3 * r:8 * j8 + 3 * r + 8, :],
                            start=False, stop=(r == 2),
                        )
                    osb = ostage.tile([128, 512], f32, tag="osb")
                    nc.vector.tensor_copy(osb, pt9)
                    nc.sync.dma_start(out=out_d[ns, :, 512 * j8:512 * (j8 + 1)], in_=osb)

    nc.compile()
    return nc


def _prep_core_inputs(xs, p2, p3, p4, w6, w7):
    """Layout-only marshaling for one core's shard xs (PER_CORE,C,H,W)."""
    xs = np.ascontiguousarray(xs, dtype=np.float32)
    xpad = np.zeros((PER_CORE, C, H, W + 6), np.float32)
    xpad[:, :, :, 3:3 + W] = xs
    # transposed blocks, reversed order, 3 zero blocks each side
    xt = xs.reshape(PER_CORE, C, S).transpose(0, 2, 1).reshape(PER_CORE, NB, 128, C)
    xtp = np.zeros((PER_CORE, 128, NBP, 128), np.float32)
    xtp[:, :, 3:3 + NB, :] = xt[:, ::-1].transpose(0, 2, 1, 3)
    return {"xpad": xpad, "xtp": xtp}


def kernel(x, p2, p3, p4, w6, w7):
    global _COMPILED
    from concourse.bass_utils import run_bass_kernel_spmd

    if _COMPILED is None:
        _COMPILED = _build_nc()
    nc = _COMPILED

    x = np.ascontiguousarray(x, dtype=np.float32)
    p2 = np.asarray(p2, dtype=np.float32)
    p3 = np.asarray(p3, dtype=np.float32)
    p4 = np.asarray(p4, dtype=np.float32)
    w6 = np.asarray(w6, dtype=np.float32)
    w7 = np.asarray(w7, dtype=np.float32)

    # shared (replicated) parameter prep — O(C*K) host work + pure layout
    p2t = np.empty((128, 128), np.float32)            # P2T[p, c] = p2[c, p%64]
    p2row = p2[0, :, 0, 0, :]                          # (C, W)
    p2t[0:64] = p2row.T
    p2t[64:128] = p2row.T
    scl = (p3[0, :, :, 0, 0] / (math.sqrt(S) * math.sqrt(7 * C))).astype(np.float32)
    w7r = np.ascontiguousarray(
        w7[:, :, 0, :].reshape(C, 7, C, 7).transpose(0, 1, 3, 2)
    )                                                  # (c2, k, sft, c'')
    dng = np.zeros((3, C, C), np.float32)
    for r in range(3):
        np.fill_diagonal(dng[r], -w6[:, 0, r, 0])
    p4p = np.ascontiguousarray(p4[0])

    shared = {"p2t": p2t, "p4p": p4p, "w7r": w7r, "scl": scl, "dng": dng}
    in_maps = []
    for i in range(N_CORES):
        m = _prep_core_inputs(x[PER_CORE * i:PER_CORE * (i + 1)], p2, p3, p4, w6, w7)
        m.update(shared)
        in_maps.append(m)

    res = run_bass_kernel_spmd(nc, in_maps, list(range(N_CORES)))
    out = np.concatenate([res.results[i]["out"] for i in range(N_CORES)], axis=0)
    return out.reshape(N, C, H, W)


# revision 9
# speedup vs baseline: 1.2585x; 1.0533x over previous
"""Trainium2 Bass kernel for nn_Kernel_11344467299061915904_53472342835846.

Reference computation (N=16, C=128, H=64, W=64, S=H*W=4096):
    t1[n,c,k,i,j] = x[n,c, i+2k-6, j]        (zero-padded in H)
    t3 = p3[c,k] * p2[c,j] * t1
    t8[n,c',(c2,k)] = sum_s x[n,c',s] t3[n,(c2,k),s] / sqrt(S)
    t7 = conv1x7(x, w7)                       (dense, 896 out channels)
    t9 = (t8 @ t7) / sqrt(7C)
    t6 = depthwise H-conv taps {-3,0,3} of roll(p4*x, 1, axis=W)
    out = t9 - t6

Restructured to cut FLOPs ~6.5x: t9 = sum_sft (t8 @ W7_sft) @ X_sft, so the
dense conv t7 is never materialized.  The H-shifts of t1 are +/-{0,2,4,6}
rows = multiples of 128 elements in (s, c) layout since 2*W = 128, so t8
becomes 32 chunk-matmuls against a block-shifted window of the transposed
input.  t6 is folded into the t9 PSUM accumulation as negated-diagonal
matmuls.  Data-parallel over batch: 2 samples per NeuronCore on 8 cores.

Host-side work is layout-only (transpose/pad/permute; zero FLOPs) plus
O(C*K) parameter prep; all O(N*C*S) arithmetic runs on device.
"""

import math

import numpy as np

N, C, H, W = 16, 128, 64, 64
S = H * W            # 4096
NB = S // 128        # 32 s-chunks of 128
NBP = NB + 6         # 38 blocks incl 3 zero pad blocks each side
PER_CORE = 2         # samples per NeuronCore
N_CORES = 8

_COMPILED = None


def _build_nc():
    import concourse.bass as bass
    import concourse.mybir as mybir
    import concourse.tile as tile
    from concourse import bacc

    f32 = mybir.dt.float32
    f32r = mybir.dt.float32r

    nc = bacc.Bacc("TRN2", target_bir_lowering=False, debug=False)

    # Per-core inputs (2 samples each), layouts pre-marshaled on host.
    xpad_d = nc.dram_tensor("xpad", [PER_CORE, C, H, W + 6], f32r, kind="ExternalInput").ap()
    xtp_d = nc.dram_tensor("xtp", [PER_CORE, 128, NBP, 128], f32r, kind="ExternalInput").ap()
    p2t_d = nc.dram_tensor("p2t", [128, 128], f32, kind="ExternalInput").ap()
    p4p_d = nc.dram_tensor("p4p", [C, H, W], f32, kind="ExternalInput").ap()
    w7r_d = nc.dram_tensor("w7r", [C, 7, 7, C], f32r, kind="ExternalInput").ap()
    scl_d = nc.dram_tensor("scl", [C, 7], f32, kind="ExternalInput").ap()
    dng_d = nc.dram_tensor("dng", [3, C, C], f32r, kind="ExternalInput").ap()
    out_d = nc.dram_tensor("out", [PER_CORE, C, S], f32, kind="ExternalOutput").ap()

    with tile.TileContext(nc) as tc:
        with (
            tc.tile_pool(name="consts", bufs=1) as consts,
            tc.tile_pool(name="xin", bufs=2) as xin,
            tc.tile_pool(name="xtr", bufs=2) as xtr,
            tc.tile_pool(name="ytr", bufs=1) as ytr,
            tc.tile_pool(name="small", bufs=1) as small,
            tc.tile_pool(name="ostage", bufs=3) as ostage,
            tc.tile_pool(name="pt8", bufs=2, space="PSUM") as pt8_pool,
            tc.tile_pool(name="pa", bufs=2, space="PSUM") as pa_pool,
            tc.tile_pool(name="pt9", bufs=2, space="PSUM") as pt9_pool,
        ):
            # ---- constants (once per core) ----
            p2t = consts.tile([128, 128], f32, tag="p2t")
            nc.sync.dma_start(out=p2t, in_=p2t_d)
            p4p = consts.tile([C, H, W], f32, tag="p4p")
            nc.gpsimd.dma_start(out=p4p, in_=p4p_d)
            w7r = consts.tile([C, 7, 7, C], f32r, tag="w7r")
            nc.gpsimd.dma_start(out=w7r, in_=w7r_d)
            scl = consts.tile([C, 7], f32, tag="scl")
            nc.sync.dma_start(out=scl, in_=scl_d)
            dng = consts.tile([C, 3, C], f32r, tag="dng")
            nc.gpsimd.dma_start(out=dng, in_=dng_d.rearrange("r p c -> p r c"))

            zcst = consts.tile([C, 3 * W], f32, tag="zcst")
            nc.vector.memset(zcst, 0.0)

            # shared across both samples: scaled transposed t8 and A matrices
            t8ts = small.tile([C, 7, PER_CORE, C], f32r, tag="t8ts")
            a_sb = small.tile([C, 7, PER_CORE, C], f32r, tag="a_sb")

            xpads = []
            t5ps = []

            # ---- per-sample: load, gate, t8 ----
            for ns in range(PER_CORE):
                xtrev = xtr.tile([128, NBP, 128], f32r, tag="xtrev")
                nc.sync.dma_start(out=xtrev[:, 28:NBP, :], in_=xtp_d[ns, :, 28:NBP, :])
                nc.sync.dma_start(out=xtrev[:, 19:28, :], in_=xtp_d[ns, :, 19:28, :])
                nc.sync.dma_start(out=xtrev[:, 9:19, :], in_=xtp_d[ns, :, 9:19, :])
                nc.sync.dma_start(out=xtrev[:, 0:9, :], in_=xtp_d[ns, :, 0:9, :])

                xpad = xin.tile([C, H, W + 6], f32r, tag="xpad")
                nc.sync.dma_start(out=xpad, in_=xpad_d[ns])
                xpads.append(xpad)

                # t5p: H-padded roll(p4*x, 1, axis=W); rows [3,67) hold data
                t5p = xin.tile([C, H + 6, W], f32r, tag="t5p")
                nc.vector.tensor_copy(t5p[:, 0:3, :], zcst.rearrange("p (a b) -> p a b", a=3))
                nc.vector.tensor_copy(t5p[:, H + 3:H + 6, :], zcst.rearrange("p (a b) -> p a b", a=3))
                # t5[c,i,j] = p4[c,i,j-1]*x[c,i,j-1]  (j>=1);  x[.,j] = xpad[., 3+j]
                nc.vector.tensor_mul(t5p[:, 3:3 + H, 1:W], xpad[:, :, 3:3 + W - 1].bitcast(f32), p4p[:, :, 0:W - 1])
                nc.vector.tensor_mul(t5p[:, 3:3 + H, 0:1], xpad[:, :, 2 + W:3 + W].bitcast(f32), p4p[:, :, W - 1:W])
                t5ps.append(t5p)

                # Yt blocks: gated transpose  Yt[m] = Xt[m] * P2T
                # xtrev block b holds X^T chunk m = 34-b  (b in [3,35))
                yt = ytr.tile([128, NB, 128], f32r, tag="yt")
                for m in range(NB):
                    nc.vector.tensor_mul(yt[:, m, :], xtrev[:, 34 - m, :].bitcast(f32), p2t)

                # t8: PT8[c2, d*128+c'] += Yt[mp].T @ XtrevBlocks[31-mp+d], d=0..6
                pt8a = pt8_pool.tile([128, 512], f32, tag="pt8a")
                pt8b = pt8_pool.tile([128, 384], f32, tag="pt8b")
                for mp in range(NB):
                    lhsT = yt[:, mp, :]
                    nc.tensor.matmul(
                        pt8a, lhsT, xtrev[:, 31 - mp:35 - mp, :],
                        start=(mp == 0), stop=(mp == NB - 1),
                    )
                    nc.tensor.matmul(
                        pt8b, lhsT, xtrev[:, 35 - mp:38 - mp, :],
                        start=(mp == 0), stop=(mp == NB - 1),
                    )

                # scaled copies into shared T8Ts[c2, d, ns, c']
                for d in range(7):
                    src = pt8a[:, 128 * d:128 * (d + 1)] if d < 4 else pt8b[:, 128 * (d - 4):128 * (d - 3)]
                    nc.vector.tensor_scalar_mul(t8ts[:, d, ns, :], src, scl[:, d:d + 1])

            # ---- A phase (both samples batched: N=256) ----
            # A_sft^T[c'', (ns, c')] = sum_k w7r[:, k, sft, :].T @ T8Ts[:, k, :, :]
            for sft in range(7):
                pa = pa_pool.tile([128, PER_CORE * 128], f32, tag="pa")
                for k in range(7):
                    nc.tensor.matmul(
                        pa, w7r[:, k, sft, :],
                        t8ts[:, k, :, :],
                        start=(k == 0), stop=(k == 6),
                    )
                nc.vector.tensor_copy(a_sb[:, sft, :, :], pa)

            # ---- t9 phase (+ folded -t6) ----
            for ns in range(PER_CORE):
                xpad, t5p = xpads[ns], t5ps[ns]
                for j8 in range(8):
                    pt9 = pt9_pool.tile([128, 512], f32, tag="pt9")
                    for sft in range(7):
                        nc.tensor.matmul(
                            pt9, a_sb[:, sft, ns, :],
                            xpad[:, 8 * j8:8 * j8 + 8, sft:sft + W],
                            start=(sft == 0), stop=False,
                        )
                    for r in range(3):
                        nc.tensor.matmul(
                            pt9, dng[:, r, :],
                            t5p[:, 8 * j8 + 3 * r:8 * j8 + 3 * r + 8, :],
                            start=False, stop=(r == 2),
                        )
                    osb = ostage.tile([128, 512], f32, tag="osb")
                    nc.vector.tensor_copy(osb, pt9)
                    nc.sync.dma_start(out=out_d[ns, :, 512 * j8:512 * (j8 + 1)], in_=osb)

    nc.compile()
    return nc


def _prep_core_inputs(xs, p2, p3, p4, w6, w7):
    """Layout-only marshaling for one core's shard xs (PER_CORE,C,H,W)."""
    xs = np.ascontiguousarray(xs, dtype=np.float32)
    xpad = np.zeros((PER_CORE, C, H, W + 6), np.float32)
    xpad[:, :, :, 3:3 + W] = xs
    # transposed blocks, reversed order, 3 zero blocks each side
    xt = xs.reshape(PER_CORE, C, S).transpose(0, 2, 1).reshape(PER_CORE, NB, 128, C)
    xtp = np.zeros((PER_CORE, 128, NBP, 128), np.float32)
    xtp[:, :, 3:3 + NB, :] = xt[:, ::-1].transpose(0, 2, 1, 3)
    return {"xpad": xpad, "xtp": xtp}


def kernel(x, p2, p3, p4, w6, w7):
    global _COMPILED
    from concourse.bass_utils import run_bass_kernel_spmd

    if _COMPILED is None:
        _COMPILED = _build_nc()
    nc = _COMPILED

    x = np.ascontiguousarray(x, dtype=np.float32)
    p2 = np.asarray(p2, dtype=np.float32)
    p3 = np.asarray(p3, dtype=np.float32)
    p4 = np.asarray(p4, dtype=np.float32)
    w6 = np.asarray(w6, dtype=np.float32)
    w7 = np.asarray(w7, dtype=np.float32)

    # shared (replicated) parameter prep — O(C*K) host work + pure layout
    p2t = np.empty((128, 128), np.float32)            # P2T[p, c] = p2[c, p%64]
    p2row = p2[0, :, 0, 0, :]                          # (C, W)
    p2t[0:64] = p2row.T
    p2t[64:128] = p2row.T
    scl = (p3[0, :, :, 0, 0] / (math.sqrt(S) * math.sqrt(7 * C))).astype(np.float32)
    w7r = np.ascontiguousarray(
        w7[:, :, 0, :].reshape(C, 7, C, 7).transpose(0, 1, 3, 2)
    )                                                  # (c2, k, sft, c'')
    dng = np.zeros((3, C, C), np.float32)
    for r in range(3):
        np.fill_diagonal(dng[r], -w6[:, 0, r, 0])
    p4p = np.ascontiguousarray(p4[0])

    shared = {"p2t": p2t, "p4p": p4p, "w7r": w7r, "scl": scl, "dng": dng}
    in_maps = []
    for i in range(N_CORES):
        m = _prep_core_inputs(x[PER_CORE * i:PER_CORE * (i + 1)], p2, p3, p4, w6, w7)
        m.update(shared)
        in_maps.append(m)

    res = run_bass_kernel_spmd(nc, in_maps, list(range(N_CORES)))
    out = np.concatenate([res.results[i]["out"] for i in range(N_CORES)], axis=0)
    return out.reshape(N, C, H, W)
